# revision 30
# baseline (speedup 1.0000x reference)
"""CommNet actor kernel for Trainium2, SPMD across 8 NeuronCores.

Math (reference):
    h      = tanh(obs @ W1 + b1)                       [N, 128]
    deg    = adj.sum(1);  msg = (adj @ h) / max(deg,1) [N, 128]
    hid    = tanh(concat(h, msg) @ W2 + b2)            [N, 128]
    logits = hid @ W3 + b3                             [N, 16]

Sharding: rows (agents) of adj are split across the 8 cores, 1024 rows each.
There are no collectives: every core recomputes the full h (134 MFLOP, cheap)
from a replicated obs, so the row-block aggregation adj[rows] @ h is fully
local.

Per-core device plan:
  E1:  full h = tanh(obs_aug @ W1_aug) in bf16 -> fp8 chunks [128, 128]
       (augmented obs carries the b1 bias row).
  E2:  hT_own = tanh(W1.T @ obsT_own + b1)  fp32r, feature-major [128, 1024]
       (the exact-precision copy of h for this core's own rows).
  AGG (fp8 DoubleRow, K=256 per matmul): h chunk-pairs are the stationary
       operand, adjT column-slices the moving operand (N=512):
         msgT_psum[r] += h[:, jp:jp+2, :].T @dr adjT_sb[:, jp:jp+2, r*512:..]
       so messages come out feature-major [128 HID, 512] directly.
       deg rides in a second DoubleRow pass with a trivial ones stationary
       -> deg_psum [1, 512].
  Normalize: rec = 1/max(deg,1) on the [1,512] deg row (DVE), broadcast
       to 128 partitions with a K=1 matmul, staged to SBUF via ACT, then
       msgT = msgT_psum * rec on DVE.
  MLP: hidT = tanh(W2h.T@hT + W2m.T@msgT + b2); logitsT = W3.T@hidT + b3
       (all fp32r feature-major); host transposes/concats the output.
       The W2h.T@hT half is issued before the aggregation so only the
       msg-dependent half sits in the tail.

  Engine-level budget measured on HW (per rep, per core): adjacency DMA
  ~27us (hard floor at ~310 GB/s/core), PE msg stream ~17us + deg stream
  ~16us + encoder ~4us, ACT ~13us (overlapped). The kernel is PE-bound;
  h_sb/hT double-buffering plus a single shared PSUM pool (8 banks:
  e1-ring 2, msg 2, deg/bc rings 2, mlp-ring 2) lets consecutive
  repetitions overlap encoder work with the previous epilogue.

adj is cast host-side to fp8 (0/1 are exact) and pre-transposed/tiled so all
DMAs are large and contiguous: 8.4 MB of adjacency per core instead of 33.5.

Build modes:
  _build_nc(reps=R)            — R unrolled repetitions (timing programs).
  _build_nc(loop=True, ...)    — single executable whose repetition count is
       a runtime input `niter` driving a hardware For_i loop; used for
       precise device-time measurement (same executable => the host/relay
       constant cancels in T(n_hi) - T(n_lo)).
"""

import numpy as np
import ml_dtypes
from contextlib import ExitStack

import concourse.tile as tile
from concourse import bacc, mybir
from concourse.bass import ts

N_AGENTS, OBS_DIM, HID, ACT_DIM = 8192, 64, 128, 16
CORES = 8
ROWS = N_AGENTS // CORES          # 1024 rows per core
JCH = N_AGENTS // 128             # 64 contraction chunks
GRP = 8                           # j-chunks per adjacency DMA (1 MiB each)

F32 = mybir.dt.float32
F32R = mybir.dt.float32r
BF16 = mybir.dt.bfloat16
FP8 = mybir.dt.float8e4
U32 = mybir.dt.uint32
BF16_NP = ml_dtypes.bfloat16
FP8_NP = ml_dtypes.float8_e4m3
FP8_ONE = 0x38  # bit pattern of 1.0 in e4m3

Tanh = mybir.ActivationFunctionType.Tanh
Identity = mybir.ActivationFunctionType.Identity

DR = mybir.MatmulPerfMode.DoubleRow
NR = ROWS // 512        # moving ranges per core
NSLAB = JCH // GRP      # adjacency slabs


def _declare_tensors(nc, loop):
    t = {}
    t["adjT"] = nc.dram_tensor("adjT", [128, JCH, ROWS], FP8,
                               kind="ExternalInput")
    t["obsTa"] = nc.dram_tensor("obsTa", [OBS_DIM + 1, N_AGENTS], BF16,
                                kind="ExternalInput")
    t["w1a"] = nc.dram_tensor("w1a", [OBS_DIM + 1, HID], BF16,
                              kind="ExternalInput")
    t["obsTo"] = nc.dram_tensor("obsTo", [OBS_DIM, ROWS], F32R,
                                kind="ExternalInput")
    t["w1"] = nc.dram_tensor("w1", [OBS_DIM, HID], F32R, kind="ExternalInput")
    t["b1"] = nc.dram_tensor("b1", [HID, 1], F32, kind="ExternalInput")
    t["w2"] = nc.dram_tensor("w2", [2, HID, HID], F32R, kind="ExternalInput")
    t["b2"] = nc.dram_tensor("b2", [HID, 1], F32, kind="ExternalInput")
    t["w3"] = nc.dram_tensor("w3", [HID, ACT_DIM], F32R, kind="ExternalInput")
    t["b3"] = nc.dram_tensor("b3", [ACT_DIM, 1], F32, kind="ExternalInput")
    if loop:
        t["niter"] = nc.dram_tensor("niter", [1, 1], U32,
                                    kind="ExternalInput")
    t["logitsT"] = nc.dram_tensor("logitsT", [ACT_DIM, ROWS], F32,
                                  kind="ExternalOutput")
    return t


def _load_consts(nc, tc, ctx, t, early_cb=None):
    """Load weights/obs into SBUF. Returns dict of const tiles.

    early_cb(oc): called after obsTa chunk `oc` is queued, so rep-0
    adjacency slabs can jump the const DMA queue (E1 only needs w1a +
    the first obsTa chunks to make progress).
    """
    consts = ctx.enter_context(tc.tile_pool(name="consts", bufs=1))
    c = {"pool": consts}

    c["w1a"] = consts.tile([OBS_DIM + 1, HID], BF16, tag="w1a", name="w1a")
    nc.sync.dma_start(c["w1a"][:], t["w1a"][:])
    # obsTa split into 8 tiles so E1 can start on chunk 0 immediately.
    OCH = 8
    ow = N_AGENTS // OCH
    c["ow"] = ow
    c["obsTa"] = []
    for oc in range(OCH):
        tl = consts.tile([OBS_DIM + 1, ow], BF16, tag=f"obsTa{oc}",
                         name=f"obsTa{oc}")
        nc.sync.dma_start(tl[:], t["obsTa"][:, oc * ow : (oc + 1) * ow])
        c["obsTa"].append(tl)
        if early_cb is not None:
            early_cb(oc)
    c["b1"] = consts.tile([HID, 1], F32, tag="b1", name="b1")
    nc.sync.dma_start(c["b1"][:], t["b1"][:])
    c["w1"] = consts.tile([OBS_DIM, HID], F32R, tag="w1", name="w1")
    nc.sync.dma_start(c["w1"][:], t["w1"][:])
    c["obsTo"] = consts.tile([OBS_DIM, ROWS], F32R, tag="obsTo", name="obsTo")
    nc.sync.dma_start(c["obsTo"][:], t["obsTo"][:])
    c["w2"] = consts.tile([HID, 2, HID], F32R, tag="w2", name="w2")
    nc.sync.dma_start(c["w2"][:], t["w2"].rearrange("c p m -> p c m"))
    c["b2"] = consts.tile([HID, 1], F32, tag="b2", name="b2")
    nc.sync.dma_start(c["b2"][:], t["b2"][:])
    c["w3"] = consts.tile([HID, ACT_DIM], F32R, tag="w3", name="w3")
    nc.sync.dma_start(c["w3"][:], t["w3"][:])
    c["b3"] = consts.tile([ACT_DIM, 1], F32, tag="b3", name="b3")
    nc.sync.dma_start(c["b3"][:], t["b3"][:])
    c["ones_dr"] = consts.tile([128, 2, 16], FP8, tag="ones_dr", name="ones_dr")
    nc.vector.memset(c["ones_dr"][:].bitcast(mybir.dt.uint8), FP8_ONE)
    c["ones_bc"] = consts.tile([1, 128], F32R, tag="ones_bc", name="ones_bc")
    nc.vector.memset(c["ones_bc"][:].bitcast(mybir.dt.uint32), 0x3F800000)
    return c


def _emit_rep(nc, tc, rctx, t, c, stage, adjp, pp, rep, early_slabs=()):
    """Emit one full repetition of the computation (encode/agg/MLP).

    Cross-rep pipelining: h_sb and hT are double-buffered and ALL PSUM
    tiles come from one shared pool `pp` whose tag rings span reps, so
    rep r+1's encoder can run while rep r's epilogue drains. Bank budget
    (8 banks): e1-ring 2, msgps 2, deg/bc shared ring 2, mlp-ring 2.
    """
    h_sb = stage.tile([128, JCH, HID], FP8, tag="h_sb", bufs=2,
                      name=f"h_sb_{rep}")
    hT = stage.tile([128, ROWS], F32R, tag="hT", bufs=2, name=f"hT_{rep}")
    msgT = stage.tile([128, ROWS], F32R, tag="msgT", name=f"msgT_{rep}")
    hidT = stage.tile([128, ROWS], F32R, tag="hidT", name=f"hidT_{rep}")
    logT = stage.tile([ACT_DIM, ROWS], F32, tag="logT", name=f"logT_{rep}")
    ow = c["ow"]

    msgps = [pp.tile([128, 512], F32, tag=f"msgps{r}",
                     name=f"msgps_{rep}_{r}") for r in range(NR)]
    # deg accumulator and the later 1/deg broadcast share one bank per
    # range (ring: deg is dead once dmax has read it).
    degps = [pp.tile([1, 512], F32, tag=f"degbc{r}",
                     name=f"degps_{rep}_{r}")[:] for r in range(NR)]

    # E1: full h, bf16 compute -> fp8, 4 chunks per psum bank so each
    # ACT eviction covers [128, 4, 128]. Emission of the 16 chunk-groups
    # is interleaved into the aggregation slab loop (see emit_agg): the
    # e1 PSUM ring (bufs=2) paces E1 matmuls to the ACT evictions, so
    # emitting them all upfront would stall the PE for ~9.6us; spread
    # between agg blocks, the PE fills those gaps with agg matmuls.
    def e1_group(q):
        ps1 = pp.tile([128, 4, HID], F32, tag="e1", bufs=2,
                      name=f"e1_{rep}_{q}")
        for k in range(4):
            j = 4 * q + k
            osb = c["obsTa"][j * 128 // ow]
            ocol = (j * 128) % ow
            nc.tensor.matmul(ps1[:, k, :], osb[:, ocol : ocol + 128],
                             c["w1a"][:], start=True, stop=True)
        nc.scalar.activation(h_sb[:, 4 * q : 4 * q + 4, :], ps1[:], Tanh)

    def e2_range(r):
        ps2 = pp.tile([128, 512], F32, tag="e1", bufs=2,
                      name=f"e2_{rep}_{r}")
        nc.tensor.matmul(ps2[:], c["w1"][:], c["obsTo"][:, ts(r, 512)],
                         start=True, stop=True)
        nc.scalar.activation(hT[:, ts(r, 512)], ps2[:], Tanh,
                             bias=c["b1"][:, 0:1])

    # Aggregation. adjT is fully resident (one slab tile per GRP chunks).
    # Emission interleaves the two ranges one slab apart: slab g carries
    # range-0 matmuls for slab g and range-1 matmuls for slab g-1, so the
    # PE has ready work while the next slab's DMA is in flight, and range
    # 0 finishes early enough that its normalize + MLP overlap the
    # range-1 drain.
    PAIRS = GRP // 2

    def agg_pairs(r, g, slab):
        # msg pairs grouped before deg pairs: fewer stationary-kind
        # switches per slab block
        for jj2 in range(PAIRS):
            j = g * GRP + 2 * jj2
            first = (g == 0 and jj2 == 0)
            last = (g == NSLAB - 1 and jj2 == PAIRS - 1)
            nc.tensor.matmul(msgps[r][:], h_sb[:, j : j + 2, :],
                             slab[:, 2 * jj2 : 2 * jj2 + 2, ts(r, 512)],
                             start=first, stop=last, perf_mode=DR)
        for jj2 in range(PAIRS):
            first = (g == 0 and jj2 == 0)
            last = (g == NSLAB - 1 and jj2 == PAIRS - 1)
            nc.tensor.matmul(degps[r], c["ones_dr"][:, :, 0:1],
                             slab[:, 2 * jj2 : 2 * jj2 + 2, ts(r, 512)],
                             start=first, stop=last, perf_mode=DR)

    # Epilogue halves, returned as closures for 2-deep rep pipelining:
    # `front` (normalize: 1/deg broadcast + msg scale, PE+DVE only) is
    # emitted after the NEXT rep's encoder; `back` (W2/W3 MLP) after the
    # next rep's aggregation. Their cross-engine latency then hides under
    # the next rep's compute instead of serializing as a tail.
    # msgT = msg_raw * (1 / max(deg, 1)); the reciprocal is taken on the
    # [1, 512] deg row BEFORE broadcasting, so the broadcast matmul
    # directly produces 1/deg and no [128,512] reciprocal is needed.
    rec1s = {}

    def norm_dve(r):
        dmax = stage.tile([1, 512], F32R, tag="dmax", name=f"dmax_{rep}_{r}")
        nc.vector.tensor_scalar_max(dmax[:], degps[r], 1.0)
        rec1 = stage.tile([1, 512], F32R, tag="rec1", name=f"rec1_{rep}_{r}")
        with nc.allow_low_precision(reason="f32r is fp32-width; layout only"):
            nc.vector.reciprocal(rec1[:], dmax[:])
        rec1s[r] = rec1

    def front():
        for r in range(NR):
            bc = pp.tile([128, 512], F32, tag=f"degbc{r}",
                         name=f"bc_{rep}_{r}")
            nc.tensor.matmul(bc[:], c["ones_bc"][:], rec1s[r][:],
                             start=True, stop=True)
            # DVE cannot read two PSUM operands; stage 1/deg through SBUF
            # on DVE (not ACT: ACT is busy with the next rep's evictions).
            bcS = stage.tile([128, 512], F32, tag="bcS",
                             name=f"bcS_{rep}_{r}")
            nc.vector.tensor_copy(bcS[:], bc[:])
            nc.vector.tensor_tensor(msgT[:, ts(r, 512)], msgps[r][:],
                                    bcS[:], mybir.AluOpType.mult)

    def back():
        for r in range(NR):
            pw = pp.tile([128, 512], F32, tag="mlp", bufs=2,
                         name=f"w2p_{rep}_{r}")
            nc.tensor.matmul(pw[:], c["w2"][:, 0, :], hT[:, ts(r, 512)],
                             start=True, stop=False)
            nc.tensor.matmul(pw[:], c["w2"][:, 1, :], msgT[:, ts(r, 512)],
                             start=False, stop=True)
            nc.scalar.activation(hidT[:, ts(r, 512)], pw[:], Tanh,
                                 bias=c["b2"][:, 0:1])
            pl = pp.tile([ACT_DIM, 512], F32, tag="mlp", bufs=2,
                         name=f"w3p_{rep}_{r}")
            nc.tensor.matmul(pl[:], c["w3"][:], hidT[:, ts(r, 512)],
                             start=True, stop=True)
            nc.scalar.activation(logT[:, ts(r, 512)], pl[:], Identity,
                                 bias=c["b3"][:, 0:1])
        nc.sync.dma_start(t["logitsT"][:], logT[:])

    def emit_agg():
        NQ = JCH // 4
        # 4-group head start (chunks 0..15 cover slabs 0-1), then two
        # groups after each slab block keeps a >=2-group lead over the
        # chunks the next slab's agg consumes.
        nq_done = 4
        for q in range(nq_done):
            e1_group(q)
        slabs = [None] * NSLAB
        for g in range(NSLAB):
            if g < len(early_slabs):
                slabs[g] = early_slabs[g]
            else:
                slabs[g] = adjp.tile([128, GRP, ROWS], FP8, tag="adjT",
                                     name=f"adjT_{rep}_{g}")
                nc.sync.dma_start(slabs[g][:],
                                  t["adjT"][:, g * GRP : (g + 1) * GRP, :])
            agg_pairs(0, g, slabs[g])
            for q in range(nq_done, min(nq_done + 2, NQ)):
                e1_group(q)
            nq_done = min(nq_done + 2, NQ)
            if g >= 1:
                agg_pairs(1, g - 1, slabs[g - 1])
        for q in range(nq_done, NQ):
            e1_group(q)
        e2_range(0)
        e2_range(1)
        norm_dve(0)              # DVE-only; overlaps the range-1 drain
        agg_pairs(1, NSLAB - 1, slabs[NSLAB - 1])
        norm_dve(1)

    return {"agg": emit_agg, "front": front, "back": back}


def _build_nc(reps=1, loop=False, unroll=1):
    nc = bacc.Bacc("TRN2", target_bir_lowering=False, debug=False,
                   num_devices=CORES)
    t = _declare_tensors(nc, loop)

    with tile.TileContext(nc) as tc, ExitStack() as ctx:
        stage = ctx.enter_context(tc.tile_pool(name="stage", bufs=1))
        adjp = ctx.enter_context(tc.tile_pool(name="adjp", bufs=NSLAB))
        pp = ctx.enter_context(tc.tile_pool(name="pp", bufs=1, space="PSUM"))

        if loop:
            nsb = ctx.enter_context(tc.tile_pool(name="nsb", bufs=1))
            niter_sb = nsb.tile([1, 1], U32, tag="niter", name="niter_sb")
            nc.sync.dma_start(niter_sb[:], t["niter"][:])
            c = _load_consts(nc, tc, ctx, t)
            n = nc.values_load(niter_sb[0:1, 0:1], min_val=0,
                               max_val=1 << 20,
                               skip_runtime_bounds_check=True)
            with tc.For_i(0, n) as _i:
                with ExitStack() as rctx:
                    prev = None
                    for u in range(unroll):
                        ph = _emit_rep(nc, tc, rctx, t, c, stage, adjp, pp,
                                       rep=u)
                        if prev is not None:
                            prev["front"]()
                        ph["agg"]()
                        if prev is not None:
                            prev["back"]()
                        prev = ph
                    prev["front"]()
                    prev["back"]()
        else:
            # rep-0 adjacency prefetch: first two slabs jump the const queue.
            early_slabs = []

            def early_cb(oc):
                if oc < 2:
                    es = adjp.tile([128, GRP, ROWS], FP8, tag="adjT",
                                   name=f"adjT_early_{oc}")
                    nc.sync.dma_start(
                        es[:], t["adjT"][:, oc * GRP : (oc + 1) * GRP, :])
                    early_slabs.append(es)

            c = _load_consts(nc, tc, ctx, t, early_cb=early_cb)
            with ExitStack() as rctx:
                prev = None
                for rep in range(reps):
                    ph = _emit_rep(nc, tc, rctx, t, c, stage, adjp, pp,
                                   rep=rep,
                                   early_slabs=early_slabs if rep == 0
                                   else ())
                    if prev is not None:
                        prev["front"]()
                    ph["agg"]()
                    if prev is not None:
                        prev["back"]()
                    prev = ph
                prev["front"]()
                prev["back"]()

    nc.compile()
    return nc


_CACHE = {}


def _get_exec(reps=1, loop=False, unroll=1):
    """Build the bass module once and wrap it in a cached jitted SPMD runner.

    This is the same execution path run_bass_kernel_spmd takes under axon
    (bass2jax._bass_exec_p -> neuronx_cc_hook -> NEFF on the 8 NeuronCores),
    but cached so repeated kernel() calls reuse the compiled executable.
    """
    key = ("exec", reps, loop, unroll)
    if key in _CACHE:
        return _CACHE[key]

    import jax
    from concourse import bass2jax

    bass2jax.install_neuronx_cc_hook()
    nc = _build_nc(reps, loop=loop, unroll=unroll)

    partition_name = (nc.partition_id_tensor.name
                      if nc.partition_id_tensor else None)
    in_names, out_names, out_avals, out_shapes = [], [], [], []
    for alloc in nc.m.functions[0].allocations:
        if not isinstance(alloc, mybir.MemoryLocationSet):
            continue
        name = alloc.memorylocations[0].name
        if alloc.kind == "ExternalInput":
            if name != partition_name:
                in_names.append(name)
        elif alloc.kind == "ExternalOutput":
            out_names.append(name)
            shape = tuple(alloc.tensor_shape)
            dtype = mybir.dt.np(alloc.dtype)
            out_avals.append(jax.core.ShapedArray(shape, dtype))
            out_shapes.append((shape, dtype))
    n_params = len(in_names)
    all_names = tuple(in_names) + tuple(out_names)
    if partition_name is not None:
        all_names = all_names + (partition_name,)

    def _step(ins, zeros):
        extra = ((bass2jax.partition_id_tensor(),)
                 if partition_name is not None else ())
        outs = bass2jax._bass_exec_p.bind(
            *ins, *zeros, *extra,
            out_avals=tuple(out_avals),
            in_names=all_names,
            out_names=tuple(out_names),
            lowering_input_output_aliases=(),
            sim_require_finite=True,
            sim_require_nnan=True,
            nc=nc,
        )
        return tuple(outs)

    devices = jax.devices()[:CORES]
    mesh = bass2jax.Mesh(np.asarray(devices), ("core",))
    spec = bass2jax.PartitionSpec("core")
    n_outs = len(out_names)
    in_specs = (spec,) * (n_params + n_outs)
    out_specs = (spec,) * n_outs if n_outs > 1 else spec

    def _body(*args):
        outs = _step(args[:n_params], args[n_params:])
        return outs if n_outs > 1 else outs[0]

    fn = jax.jit(bass2jax.shard_map(
        _body, mesh=mesh, in_specs=in_specs, out_specs=out_specs,
        check_rep=False))

    _CACHE[key] = dict(nc=nc, fn=fn, mesh=mesh,
                       spec=spec, in_names=in_names, out_names=out_names,
                       out_shapes=out_shapes, n_params=n_params)
    return _CACHE[key]


def _prep_in_maps(inputs, niter=None):
    obs = np.asarray(inputs["obs_agents"], np.float32)
    adj = np.asarray(inputs["adj"])
    W1 = np.asarray(inputs["W1"], np.float32)
    b1 = np.asarray(inputs["b1"], np.float32)
    W2 = np.asarray(inputs["W2"], np.float32)
    b2 = np.asarray(inputs["b2"], np.float32)
    W3 = np.asarray(inputs["W3"], np.float32)
    b3 = np.asarray(inputs["b3"], np.float32)

    obsT = np.ascontiguousarray(obs.T)                       # [64, 8192]
    obsTa = np.concatenate(
        [obsT, np.ones((1, N_AGENTS), np.float32)], axis=0).astype(BF16_NP)
    w1a = np.concatenate([W1, b1[None, :]], axis=0).astype(BF16_NP)
    w2c = np.ascontiguousarray(W2.reshape(2, HID, HID))
    b1c = np.ascontiguousarray(b1.reshape(HID, 1))
    b2c = np.ascontiguousarray(b2.reshape(HID, 1))
    b3c = np.ascontiguousarray(b3.reshape(ACT_DIM, 1))
    w3c = np.ascontiguousarray(W3)

    # adjacency 0/1 -> fp8 bit pattern, then per-core transpose + chunk tiling
    adj_u8 = adj.astype(np.uint8) * np.uint8(FP8_ONE)

    in_maps = []
    for cidx in range(CORES):
        r0 = cidx * ROWS
        adjTc = np.ascontiguousarray(
            adj_u8[r0 : r0 + ROWS].T.reshape(JCH, 128, ROWS)
            .transpose(1, 0, 2)).view(FP8_NP)
        obsTo = np.ascontiguousarray(obsT[:, r0 : r0 + ROWS])
        m = {
            "adjT": adjTc, "obsTa": obsTa, "w1a": w1a, "obsTo": obsTo,
            "w1": W1, "b1": b1c, "w2": w2c, "b2": b2c, "w3": w3c, "b3": b3c,
        }
        if niter is not None:
            m["niter"] = np.array([[niter]], np.uint32)
        in_maps.append(m)
    return in_maps


def _concat_args(ex, in_maps):
    concat_in = [
        np.concatenate([in_maps[c][nm] for c in range(CORES)], axis=0)
        for nm in ex["in_names"]
    ]
    concat_zeros = [
        np.zeros((CORES * shape[0], *shape[1:]), dtype)
        for shape, dtype in ex["out_shapes"]
    ]
    return concat_in, concat_zeros


def _unshard_logits(ex, out_arr):
    lt = np.asarray(out_arr).reshape(CORES, ACT_DIM, ROWS)
    out = np.empty((N_AGENTS, ACT_DIM), np.float32)
    for c in range(CORES):
        out[c * ROWS : (c + 1) * ROWS] = lt[c].T
    return out


def run(inputs):
    in_maps = _prep_in_maps(inputs)
    try:
        ex = _get_exec()
        concat_in, concat_zeros = _concat_args(ex, in_maps)
        out_arr = ex["fn"](*concat_in, *concat_zeros)
        return _unshard_logits(ex, out_arr)
    except Exception:
        # Fallback: the stock SPMD runner (same execution path, uncached).
        from concourse.bass_utils import run_bass_kernel_spmd
        if "nc" not in _CACHE:
            _CACHE["nc"] = _build_nc()
        res = run_bass_kernel_spmd(_CACHE["nc"], in_maps, list(range(CORES)))
        out = np.empty((N_AGENTS, ACT_DIM), np.float32)
        for c in range(CORES):
            out[c * ROWS : (c + 1) * ROWS] = res.results[c]["logitsT"].T
        return out


def loop_timed_run(inputs, n_lo=2, n_hi=64, calls=30, unroll=1):
    """Device-exact timing via the hardware-loop executable.

    One executable, runtime iteration count => the per-call host/relay
    constant is identical for n_lo and n_hi and cancels in the difference:
        per_rep = (minT(n_hi) - minT(n_lo)) / ((n_hi - n_lo) * unroll)
    Returns (output_of_1_iter, per_rep_ns).
    """
    import jax
    import time

    ex = _get_exec(loop=True, unroll=unroll)
    sharding = jax.sharding.NamedSharding(ex["mesh"], ex["spec"])
    base_maps = _prep_in_maps(inputs, niter=0)

    def dev_args(niter):
        for m in base_maps:
            m["niter"] = np.array([[niter]], np.uint32)
        concat_in, concat_zeros = _concat_args(ex, base_maps)
        return ([jax.device_put(a, sharding) for a in concat_in],
                [jax.device_put(z, sharding) for z in concat_zeros])

    args = {n: dev_args(n) for n in (1, n_lo, n_hi)}

    def sample(n, k):
        di, dz = args[n]
        out = jax.block_until_ready(ex["fn"](*di, *dz))
        best = float("inf")
        for _ in range(k):
            t0 = time.perf_counter()
            out = ex["fn"](*di, *dz)
            jax.block_until_ready(out)
            best = min(best, time.perf_counter() - t0)
        return best, out

    mins = {n_lo: float("inf"), n_hi: float("inf")}
    out1 = None
    BLK = 10
    for _ in range(max(1, calls // BLK)):
        for n in (n_lo, n_hi):
            b, out = sample(n, BLK)
            mins[n] = min(mins[n], b)
    _, out1 = sample(1, 3)
    per_rep_ns = (mins[n_hi] - mins[n_lo]) / ((n_hi - n_lo) * unroll) * 1e9
    return _unshard_logits(ex, out1), per_rep_ns


def timed_run(inputs, reps=16, iters=20, rounds=4):
    """Two-point device timing. The per-call RPC overhead (~4 ms under the
    axon relay) hides small device times, so we also build a program that
    repeats the whole kernel `reps` times on-device and report
    (T_reps - T_1) / (reps - 1), which isolates the true steady-state
    per-invocation device time. Returns (output, per_rep_ns).
    """
    import jax, time

    def bench(ex, dev_in, dev_zeros):
        fn = ex["fn"]
        out = jax.block_until_ready(fn(*dev_in, *dev_zeros))
        best = float("inf")
        for _ in range(rounds):
            t0 = time.perf_counter()
            for _ in range(iters):
                out = fn(*dev_in, *dev_zeros)
            jax.block_until_ready(out)
            best = min(best, (time.perf_counter() - t0) / iters)
        return best, out

    ex1 = _get_exec(reps=1)
    concat_in, concat_zeros = _concat_args(ex1, _prep_in_maps(inputs))
    sharding = jax.sharding.NamedSharding(ex1["mesh"], ex1["spec"])
    dev_in = [jax.device_put(a, sharding) for a in concat_in]
    dev_zeros = [jax.device_put(z, sharding) for z in concat_zeros]
    exR = _get_exec(reps=reps)
    # several alternating paired measurements; dispatch-noise outliers are
    # rejected by the median. The relay noise floor is large relative to
    # the device time, so take many pairs for a stable median.
    estimates = []
    out1 = outR = None
    for _ in range(11):
        t1, out1 = bench(ex1, dev_in, dev_zeros)
        tR, outR = bench(exR, dev_in, dev_zeros)
        estimates.append((tR - t1) / (reps - 1) * 1e9)
    ref = _unshard_logits(ex1, out1)
    chk = _unshard_logits(exR, outR)
    if not np.allclose(ref, chk, rtol=1e-5, atol=1e-6):
        print("WARNING: reps-program output mismatch; timing suspect")
    per_rep_ns = float(np.median(estimates))
    print("two-point per-rep estimates (ns):",
          [f"{e:.0f}" for e in estimates])
    return ref, per_rep_ns


def kernel(**inputs) -> np.ndarray:
    return run(inputs)
